# revision 9
# baseline (speedup 1.0000x reference)
"""BFP (block-floating-point) quantized linear on 8 TRN2 NeuronCores.

out = quantize_bfp(x) @ quantize_bfp(weight).T + bias
  - groups of 32 contiguous elements along the contraction dim share one
    exponent e = floor(log2(max_abs)); scale s = 2^(e-7);
    q = clip(round(v/s), -128, 127) * s  (round half-to-even), zero-guarded.

Key facts this kernel exploits:
  * Quantized values are k * 2^(e-7) with |k| <= 128 -> exactly representable
    in bf16, so the matmul runs at bf16 TensorE peak with no extra error.
  * round-to-multiple-of-s == (v + 1.5*2^23*s) - 1.5*2^23*s in f32 RN
    (half-to-even, matching jnp.round).
  * clip bounds are derived from C = 1.5*2^23*s by two multiplies:
    hi = C * (127/12582912) ~= 127*s, lo = C * (-1/98304) ~= -128*s; the
    ~2^-24 relative error on the bounds is absorbed by the bf16 output cast.
  * The whole apply (round + clip) is ONE fused custom DVE instruction
    (5 ALU stages), with the per-group C broadcast via a stride-0 inner dim.

Sharding: 2 x 4 grid (M split by 2, OUT split by 4). Every core runs the
same program on x[4096,4096], w[1024,4096], b[1024] -> out[4096,1024].
"""

import numpy as np

import concourse.bass as bass
import concourse.tile as tile
from concourse import bacc, mybir
from concourse._compat import with_exitstack
from concourse.bass_utils import run_bass_kernel_spmd

DT = mybir.dt

# Problem shape (hardcoded per contest contract)
M, IN, OUT = 8192, 4096, 4096
PM, PN = 2, 4
M_SH, N_SH = M // PM, OUT // PN  # 4096, 1024 per core
GS = 32          # bfp group size
P = 128          # partitions
NT = 512         # matmul moving free dim (one PSUM bank of f32)

# magic constants for the fused round+clip
_C_MUL = 98304.0              # 1.5 * 2^16: C = 2^e * _C_MUL = 1.5*2^23*s
_HI_K = 127.0 / 12582912.0    # C * _HI_K = 127 * s
_LO_K = -1.0 / 98304.0        # C * _LO_K = -128 * s
_EXP_MASK = 0x7F800000
_EXP_MIN = 0x00800000         # clamp exponent field >= 1 (zero-group guard)

# ---------------------------------------------------------------------------
# custom fused DVE op: out = clip(round_to_multiple(x, s), -128s, 127s)
# in0 = x, in1 = C (= 1.5*2^23*s) broadcast per group, s0 = _HI_K, s1 = _LO_K
# ---------------------------------------------------------------------------
_BFP_OP = None


def _bfp_apply_ref(in0, in1, c0, c1, c2):
    x = np.asarray(in0, np.float32)
    C = np.asarray(in1, np.float32).reshape(x.shape)
    t = ((x + C).astype(np.float32) - C).astype(np.float32)
    hi = (C * np.float32(c0)).astype(np.float32)
    lo = (C * np.float32(c1)).astype(np.float32)
    return np.maximum(np.minimum(t, hi), lo)


def get_bfp_op():
    global _BFP_OP
    if _BFP_OP is not None:
        return _BFP_OP
    from concourse.dve_ops import (
        CUSTOM_DVE_SPECS,
        OPS,
        _CUSTOM_DVE_ROW_BASE,
        _SUB_OPCODE_FOR_NAME,
        DveOp,
    )
    from concourse.dve_spec import C0, C1, Spec, Src0, Src1, lower, maxx, minn
    from concourse.dve_uop import DveOpSpec

    for existing in OPS:
        if existing.name == "BFP_APPLY_ANT":
            _BFP_OP = existing
            return existing

    t = (Src0 + Src1) - Src1
    spec = Spec(
        body=maxx(minn(t, Src1 * C0), Src1 * C1),
        reference=_bfp_apply_ref,
    )
    shas = {
        ver: DveOpSpec(
            name="BFP_APPLY_ANT", uops=lower(spec, ver=ver), rd1_en=True
        ).sha(ver)
        for ver in ("v3", "v4")
    }
    op = DveOp("BFP_APPLY_ANT", spec, subdim=False, uops_sha=shas)
    OPS.append(op)
    CUSTOM_DVE_SPECS[op.name] = op.spec
    _SUB_OPCODE_FOR_NAME[op.name] = _CUSTOM_DVE_ROW_BASE + len(OPS) - 1
    _BFP_OP = op
    return op


# ---------------------------------------------------------------------------
# Tile kernel builder (parameterized so it can be sim-tested at small sizes)
# ---------------------------------------------------------------------------
@with_exitstack
def build_bfl(ctx, tc, out_ap, x_ap, w_ap, b_ap, m_sh, n_sh, k):
    nc = tc.nc
    op = get_bfp_op()
    G = k // GS        # groups per row
    KC = k // P        # 128-wide k-chunks
    n_wt = n_sh // P   # weight row-tiles
    n_mt = m_sh // P   # x row-tiles
    n_nt = n_sh // NT  # psum column strips
    wt_per_nt = NT // P

    stage = ctx.enter_context(tc.tile_pool(name="stage", bufs=3))
    qpool = ctx.enter_context(tc.tile_pool(name="q", bufs=3))
    qtpool = ctx.enter_context(tc.tile_pool(name="qt", bufs=3))
    gpool = ctx.enter_context(tc.tile_pool(name="g", bufs=2))
    wqt_pool = ctx.enter_context(tc.tile_pool(name="wqt", bufs=1))
    cpool = ctx.enter_context(tc.tile_pool(name="const", bufs=1))
    opool = ctx.enter_context(tc.tile_pool(name="o", bufs=4))
    pspool = ctx.enter_context(tc.tile_pool(name="ps", bufs=4, space="PSUM"))

    # bias is added by seeding PSUM with a K=2 bf16 matmul: ones.T @ [b_hi; b_lo]
    # where b = b_hi + b_lo (bf16 hi/lo split, residual ~2^-18 relative).
    ones_t = cpool.tile([2, P], DT.bfloat16, tag="ones")
    nc.vector.memset(ones_t[:], 1.0)
    bias_f = cpool.tile([1, n_sh], DT.float32, tag="bias_f")
    nc.sync.dma_start(bias_f[:], b_ap.unsqueeze(0))
    bias_t = cpool.tile([2, n_sh], DT.bfloat16, tag="bias")
    nc.vector.tensor_copy(bias_t[0:1, :], bias_f[:])
    bias_lo = cpool.tile([1, n_sh], DT.bfloat16, tag="bias_lo")
    nc.vector.tensor_tensor(
        bias_lo[:], bias_f[:], bias_t[0:1, :], op=mybir.AluOpType.subtract
    )
    nc.sync.dma_start(bias_t[1:2, :], bias_lo[:])

    def quantize(src_dram, qtile):
        """DMA a [P, k] f32 tile and BFP-quantize it into bf16 qtile."""
        xt = stage.tile([P, k], DT.float32, tag="stage")
        nc.scalar.dma_start(xt[:], src_dram)
        gm = gpool.tile([P, G], DT.float32, tag="gmax")
        nc.vector.tensor_reduce(
            gm[:],
            xt[:].rearrange("p (g j) -> p g j", j=GS),
            axis=mybir.AxisListType.X,
            op=mybir.AluOpType.max,
            apply_absolute_value=True,
        )
        ci = gpool.tile([P, G], DT.int32, tag="ci")
        nc.vector.tensor_scalar(
            ci[:],
            gm[:].bitcast(DT.int32),
            _EXP_MASK,
            None,
            op0=mybir.AluOpType.bitwise_and,
        )
        nc.vector.tensor_scalar_max(ci[:], ci[:], _EXP_MIN)
        cf = gpool.tile([P, G], DT.float32, tag="cf")
        nc.vector.tensor_scalar_mul(cf[:], ci[:].bitcast(DT.float32), _C_MUL)
        nc.vector._custom_dve(
            op,
            out=qtile[:],
            in0=xt[:],
            in1=cf[:].unsqueeze(2).broadcast_to([P, G, GS]),
            s0=_HI_K,
            s1=_LO_K,
        )

    # ---- W phase: quantize weight shard, transpose into resident wqT ----
    # Interleaved with the first x tiles so nt=0 matmuls can start as soon
    # as W-tiles 0..wt_per_nt-1 are done (wqt[nt] dep is per-tile).
    wqt = [
        wqt_pool.tile([P, KC * NT], DT.bfloat16, tag=f"wqt{i}", name=f"wqt{i}")
        for i in range(n_nt)
    ]

    def do_w_tile(wt):
        wq = qpool.tile([P, k], DT.bfloat16, tag="q", name=f"wq{wt}")
        quantize(w_ap[wt * P : (wt + 1) * P, :], wq)
        nt, col = wt // wt_per_nt, (wt % wt_per_nt) * P
        dst = wqt[nt][:].rearrange("p (c n) -> p c n", n=NT)[:, :, col : col + P]
        nc.sync.dma_start_transpose(dst, wq[:])

    def do_x_quant(mt):
        xq = qpool.tile([P, k], DT.bfloat16, tag="q", name=f"xq{mt}")
        quantize(x_ap[mt * P : (mt + 1) * P, :], xq)
        xqt = qtpool.tile([P, KC * P], DT.bfloat16, tag="xqt", name=f"xqt{mt}")
        xqt3 = xqt[:].rearrange("p (c m) -> p c m", m=P)
        nc.sync.dma_start_transpose(xqt3, xq[:])
        return xqt3

    for wt in range(wt_per_nt):
        do_w_tile(wt)
    xqt3_0 = do_x_quant(0)
    for wt in range(wt_per_nt, n_wt):
        do_w_tile(wt)

    # ---- X phase: per 128-row x tile: quantize, transpose, matmul ----
    for mt in range(n_mt):
        xqt3 = xqt3_0 if mt == 0 else do_x_quant(mt)
        for nt in range(n_nt):
            ps = pspool.tile([P, NT], DT.float32, tag="ps")
            # bias seeds the accumulator via a K=1 f32 matmul of ones x bias
            nc.tensor.matmul(
                ps[:],
                lhsT=ones_t[:],
                rhs=bias_t[:, nt * NT : (nt + 1) * NT],
                start=True,
                stop=False,
            )
            wq3 = wqt[nt][:].rearrange("p (c n) -> p c n", n=NT)
            for c in range(KC):
                nc.tensor.matmul(
                    ps[:],
                    lhsT=xqt3[:, c, :],
                    rhs=wq3[:, c, :],
                    start=False,
                    stop=(c == KC - 1),
                )
            ob = opool.tile([P, NT], DT.float32, tag="o")
            nc.scalar.copy(ob[:], ps[:])
            nc.scalar.dma_start(
                out_ap[mt * P : (mt + 1) * P, nt * NT : (nt + 1) * NT], ob[:]
            )


# ---------------------------------------------------------------------------
# host entry
# ---------------------------------------------------------------------------
_CACHE = {}
LAST_EXEC_NS = None
LAST_RESULTS = None


def _build(m_sh, n_sh, k, num_devices=8):
    key = (m_sh, n_sh, k)
    if key in _CACHE:
        return _CACHE[key]
    nc = bacc.Bacc(
        "TRN2",
        target_bir_lowering=False,
        debug=False,
        enable_asserts=True,
        num_devices=num_devices,
    )
    x_ap = nc.dram_tensor("x", [m_sh, k], DT.float32, kind="ExternalInput").ap()
    w_ap = nc.dram_tensor("w", [n_sh, k], DT.float32, kind="ExternalInput").ap()
    b_ap = nc.dram_tensor("b", [n_sh], DT.float32, kind="ExternalInput").ap()
    out_ap = nc.dram_tensor(
        "out", [m_sh, n_sh], DT.float32, kind="ExternalOutput"
    ).ap()
    with tile.TileContext(nc) as tc:
        build_bfl(tc, out_ap, x_ap, w_ap, b_ap, m_sh, n_sh, k)
    nc.compile()
    _CACHE[key] = nc
    return nc


def _install_ntff_hook():
    """This image lacks ``antenv.axon_hooks``; synthesize it so
    run_bass_kernel_spmd's trace path can ship NTFFs back via ctypes."""
    import sys
    import types

    if "antenv.axon_hooks" in sys.modules:
        return
    try:
        from trn_agent_boot.trn_boot import _ntff_profile_via_ctypes

        hook = _ntff_profile_via_ctypes("/opt/axon/libaxon_pjrt.so")
    except Exception:
        hook = None
    mod = types.ModuleType("antenv.axon_hooks")
    state = {"hook": hook}
    mod.get_axon_ntff_profile_hook = lambda: state["hook"]
    mod.set_axon_ntff_profile_hook = lambda h: state.update(hook=h)
    sys.modules["antenv.axon_hooks"] = mod


def kernel(x, weight, bias, trace=False):
    global LAST_EXEC_NS, LAST_RESULTS
    if trace:
        _install_ntff_hook()
    x = np.ascontiguousarray(np.asarray(x, np.float32))
    weight = np.ascontiguousarray(np.asarray(weight, np.float32))
    bias = np.ascontiguousarray(np.asarray(bias, np.float32))
    assert x.shape == (M, IN) and weight.shape == (OUT, IN) and bias.shape == (OUT,)

    nc = _build(M_SH, N_SH, IN)
    in_maps = []
    for c in range(8):
        mb, nb = c // PN, c % PN
        in_maps.append(
            {
                "x": np.ascontiguousarray(x[mb * M_SH : (mb + 1) * M_SH]),
                "w": np.ascontiguousarray(weight[nb * N_SH : (nb + 1) * N_SH]),
                "b": np.ascontiguousarray(bias[nb * N_SH : (nb + 1) * N_SH]),
            }
        )
    res = run_bass_kernel_spmd(nc, in_maps, core_ids=list(range(8)), trace=trace)
    LAST_EXEC_NS = res.exec_time_ns
    LAST_RESULTS = res
    out = np.empty((M, OUT), np.float32)
    for c in range(8):
        mb, nb = c // PN, c % PN
        out[mb * M_SH : (mb + 1) * M_SH, nb * N_SH : (nb + 1) * N_SH] = res.results[
            c
        ]["out"]
    return out
